# revision 1
# baseline (speedup 1.0000x reference)
"""CAM (channel attention module) kernel for Trainium2, 8-core SPMD.

Problem: x (16, 512, 64, 64) f32, gamma (1,) f32.
  v = x.reshape(B, C, N);  E = v @ v.T  (B x 512 x 512)
  att = softmax(rowmax(E) - E)  ==  exp(rowmin(E) - E) / rowsum(...)
  out = gamma * (att @ v) + x

Sharding: data-parallel over batch, 2 batches per core, no collectives.

Per-core per-batch pipeline (energy in fp16 = TF32-class softmax logits;
attention applied in fp8e4m3 + a residual-correction pass ~= fp16
accuracy; the x residual rides on the fp16 copy of x, ~4e-4 rel):
  T: v16 (fp16) and v8 (fp8e4m3) quarter tiles are loaded straight
     from HBM with gpsimd casting DMAs (the SWDGE converts dtype in
     flight, so no f32 loads and no on-chip convert pass exist at
     all); 128 PE transposes (fp16) + DVE/ACT copies build the
     per-quarter vT tiles [128, 8, 4, 128]; dv8 = v16 - v8 residual
     tiles are built on DVE/gpsimd while the PE runs the energy
     matmuls.
  E: energy = vT.T @ vT, fp16 operands, f32 PSUM accumulation. Symmetry:
     row-tile ct computes only columns >= ct*128; the 6 missing blocks
     are mirrored from earlier rows by PE transpose into the PSUM row.
  S: rowmin (DVE), then one ACT exp with accumulated f32 rowsum writing
     the UNSCALED attention weights directly as fp8e4m3 (max entry is
     exactly 1.0 per row); gamma/rowsum stays a per-row f32 scalar rg
     applied in the epilogue, so gamma==0 zeroes the attention term.
  A: 16 PE transposes att8 -> attT pairs (fp8), per dt-pair tiles.
  O: out = attT.T @ v8 + attT.T @ dv8 with fp8 DoubleRow matmuls (0.5
     PE cycles per output row, contracting 256 channels per
     instruction, 4x the fp16 matmul rate) into f32 PSUM -- the two
     256-wide halves of each output tile accumulate as sequential
     groups in one full PSUM bank; the epilogue fuses
     (psum * rg) + x16 in one DVE scalar_tensor_tensor per [128, 512]
     tile (gpsimd cannot read PSUM on hardware); 256KB stores on the
     sync/scalar queues.
"""
import sys

import numpy as np

if "/opt/trn_rl_repo" not in sys.path:
    sys.path.insert(0, "/opt/trn_rl_repo")

import concourse.bass as bass
import concourse.tile as tile
from concourse import bacc, mybir
from concourse.bass_utils import run_bass_kernel_spmd
from concourse.masks import make_identity

N_CORES = 8
B_FULL = 16
B_PER_CORE = B_FULL // N_CORES  # 2
C = 512            # channels
HW = 4096          # H*W
CT = C // 128      # 4 channel tiles
QW = HW // 4       # quarter of H*W
NCH = HW // 512    # 8 output column chunks (512 wide)

f32 = mybir.dt.float32
f16 = mybir.dt.float16
f8 = mybir.dt.float8e4

_CACHE = {}

DR = mybir.MatmulPerfMode.DoubleRow


def _build_nc():
    nc = bacc.Bacc(None, target_bir_lowering=False)
    x_d = nc.dram_tensor("x", [B_PER_CORE, C, HW], f32, kind="ExternalInput")
    g_d = nc.dram_tensor("gamma", [1], f32, kind="ExternalInput")
    y_d = nc.dram_tensor("y", [B_PER_CORE, C, HW], f32, kind="ExternalOutput")

    with tile.TileContext(nc) as tc:
        with (
            tc.tile_pool(name="pxq", bufs=2) as pxq,        # f32 x staging
            tc.tile_pool(name="pv16", bufs=2) as pv16,      # fp16 v quarters
            tc.tile_pool(name="pvt", bufs=2) as pvt,        # vT quarter tiles
            tc.tile_pool(name="pv8", bufs=1) as pv8,        # fp8 v / dv pairs
            tc.tile_pool(name="patt", bufs=1) as patt,      # att8 / attT8
            tc.tile_pool(name="pstage", bufs=4) as pstage,  # out staging
            tc.tile_pool(name="psmall", bufs=8) as psmall,  # per-ct scalars
            tc.tile_pool(name="pmir", bufs=1) as pmir,      # mirror blocks
            tc.tile_pool(name="psing", bufs=1) as psing,    # ident, gamma
            tc.tile_pool(name="ptp", bufs=2, space="PSUM") as ptp,
            tc.tile_pool(name="pep", bufs=2, space="PSUM") as pep,
            tc.tile_pool(name="pop", bufs=3, space="PSUM") as pop,
            tc.tile_pool(name="pat", bufs=1, space="PSUM") as pat,
        ):
            ident = psing.tile([128, 128], f32)
            make_identity(nc, ident)
            ident16 = psing.tile([128, 128], f16)
            nc.vector.tensor_copy(out=ident16, in_=ident)
            gam = psing.tile([128, 1], f32)

            def load_gamma():
                g_ap = g_d[:]
                nc.gpsimd.dma_start(
                    out=gam,
                    in_=bass.AP(tensor=g_ap.tensor, offset=g_ap.offset,
                                ap=[[0, 128], [1, 1]]),
                )

            def load_v16(b):
                """fp16 casting loads of x straight from HBM (the gpsimd
                DGE converts f32->f16 in flight, halving load bytes and
                removing the convert pass), then PE transposes into vT.
                Consumers are engine ops, so the Pool DMA queue pipelines.
                """
                tiles = [[None] * 4 for _ in range(CT)]
                v8p = v8_tiles()
                vTq = [pvt.tile([128, 8, CT, 128], f16, tag=f"vTq{q}",
                                name=f"vTq{q}") for q in range(4)]
                for i, (ct, q) in enumerate(
                        (ct, q) for ct in range(CT) for q in range(4)):
                    t_ = pv16.tile([128, QW], f16, tag=f"v16_{ct}q{q}",
                                   name=f"v16_{ct}q{q}")
                    nc.gpsimd.dma_start(
                        out=t_,
                        in_=x_d[b, ct * 128:(ct + 1) * 128,
                                q * QW:(q + 1) * QW],
                    )
                    tp = ptp.tile([128, 8, 128], f16, tag="tp")
                    for ks in range(8):
                        nc.tensor.transpose(
                            tp[:, ks, :],
                            t_[:, ks * 128:(ks + 1) * 128],
                            ident16,
                        )
                    if i % 2:
                        nc.vector.tensor_copy(out=vTq[q][:, :, ct, :],
                                              in_=tp)
                    else:
                        nc.scalar.copy(out=vTq[q][:, :, ct, :], in_=tp)
                    tiles[ct][q] = t_
                # fp8 v via gpsimd casting loads straight from HBM
                for g in range(2):
                    for j in range(2):
                        for q in range(4):
                            load_v8_slice(b, v8p, g, j, q)
                return tiles, vTq, v8p

            def v8_tiles():
                return [pv8.tile([128, 2, HW], f8, tag=f"v8_{g}",
                                 name=f"v8_{g}") for g in range(2)]

            def load_v8_slice(b, v8p, g, j, q):
                """fp8 casting load of one quarter into the (g, j) row pair."""
                ct = 2 * g + j
                nc.gpsimd.dma_start(
                    out=v8p[g][:, j, q * QW:(q + 1) * QW],
                    in_=x_d[b, ct * 128:(ct + 1) * 128, q * QW:(q + 1) * QW],
                )

            batches = list(range(B_PER_CORE))
            v16, vTq, v8p = load_v16(batches[0])
            load_gamma()

            v16_next = vTq_next = v8p_next = None

            for bi, b in enumerate(batches):
                if bi > 0:
                    v16, vTq, v8p = v16_next, vTq_next, v8p_next

                def vT(k):
                    return vTq[k // 8][:, k % 8, :, :]

                # dv8 residual pair tiles (filled on DVE during the E phase)
                dv8p = [pv8.tile([128, 2, HW], f8, tag=f"dv8_{g}",
                                 name=f"dv8_{g}") for g in range(2)]

                def emit_dv8(ct, q):
                    g, j = divmod(ct, 2)
                    eng = nc.vector if q == 0 else nc.gpsimd
                    eng.tensor_sub(
                        out=dv8p[g][:, j, q * QW:(q + 1) * QW],
                        in0=v16[ct][q],
                        in1=v8p[g][:, j, q * QW:(q + 1) * QW],
                    )

                # ---- E + S: energy (fp16, f32 accum) + fp8 softmax ----
                att8 = [None] * CT
                rg2 = [None] * CT
                mirror_src = {}
                for ct in range(CT):
                    off = ct * 128
                    ep = pep.tile([128, C], f32, tag="ep")
                    for k in range(32):
                        nc.tensor.matmul(
                            ep[:, off:],
                            lhsT=vT(k)[:, ct, :],
                            rhs=vT(k)[:, ct:, :],
                            start=(k == 0),
                            stop=(k == 31),
                        )
                    for (dst, src) in (((1, 0), (0, 1)), ((2, 0), (0, 2)),
                                       ((2, 1), (1, 2)), ((3, 0), (0, 3)),
                                       ((3, 1), (1, 3)), ((3, 2), (2, 3))):
                        if src[0] == ct:
                            sb = pmir.tile([128, 128], f32,
                                           tag=f"mir{dst[0]}{dst[1]}")
                            nc.vector.tensor_copy(
                                out=sb,
                                in_=ep[:, src[1] * 128:(src[1] + 1) * 128],
                            )
                            mirror_src[dst] = sb
                    for dt in range(ct):
                        nc.tensor.transpose(
                            ep[:, dt * 128:(dt + 1) * 128],
                            mirror_src[(ct, dt)], ident,
                        )
                    mn = psmall.tile([128, 1], f32, tag="mn")
                    nc.vector.tensor_reduce(
                        out=mn, in_=ep, axis=mybir.AxisListType.X,
                        op=mybir.AluOpType.min,
                    )
                    a_ = patt.tile([128, C], f16, tag=f"att{ct}")
                    ss = psmall.tile([128, 1], f32, tag="ss")
                    nc.scalar.activation(
                        out=a_, in_=ep,
                        func=mybir.ActivationFunctionType.Exp,
                        bias=mn, scale=-1.0, accum_out=ss,
                    )
                    rg = psmall.tile([128, 1], f32, tag=f"rg{ct}")
                    nc.vector.reciprocal(out=rg, in_=ss)
                    nc.vector.tensor_mul(out=rg, in0=rg, in1=gam)
                    att8[ct] = a_
                    rg2[ct] = rg
                    # residual tiles built while the PE is busy on E
                    for q in range(4):
                        emit_dv8(ct, q)

                # ---- A: transpose att8 -> attT pairs (fp8) ----
                attT = [patt.tile([128, 2, CT, 128], f8, tag=f"attT{g}",
                                  name=f"attT{g}") for g in range(2)]
                for dt in range(CT):
                    atp = pat.tile([128, CT, 128], f16, tag="atp",
                                   name=f"atp{dt}")
                    for ct in range(CT):
                        nc.tensor.transpose(
                            atp[:, ct, :],
                            att8[ct][:, dt * 128:(dt + 1) * 128],
                            ident16,
                        )
                    nc.vector.tensor_copy(
                        out=attT[dt // 2][:, dt % 2, :, :], in_=atp,
                    )

                # prefetch next batch's full T chain (loads + converts +
                # transposes + fp8 casts) after the A phase
                if bi + 1 < len(batches):
                    v16_next, vTq_next, v8p_next = load_v16(batches[bi + 1])
                else:
                    v16_next = vTq_next = v8p_next = None

                # ---- O: DoubleRow fp8 out = attT.T @ (v8 + dv8) ----
                for n in range(NCH):
                    nsl = slice(n * 512, (n + 1) * 512)
                    for ct in range(CT):
                        # one full-bank psum per (n, ct): the two 256-wide
                        # DoubleRow groups run sequentially into its halves
                        op = pop.tile([128, 2, 256], f32, tag="op")
                        for h in range(2):
                            hsl = slice(n * 512 + h * 256,
                                        n * 512 + (h + 1) * 256)
                            for pi, src in enumerate((v8p, dv8p)):
                                for g in range(2):
                                    nc.tensor.matmul(
                                        op[:, h, :],
                                        lhsT=attT[g][:, :, ct, :],
                                        rhs=src[g][:, :, hsl],
                                        start=(pi == 0 and g == 0),
                                        stop=(pi == 1 and g == 1),
                                        perf_mode=DR,
                                    )
                        st = pstage.tile([128, 512], f32, tag="st")
                        # gpsimd cannot read PSUM on hardware: the fused
                        # epilogue always runs on DVE
                        nc.vector.scalar_tensor_tensor(
                            out=st,
                            in0=op,
                            scalar=rg2[ct],
                            in1=v16[ct][2 * n // 4][
                                :, (n * 512) % QW:(n * 512) % QW + 512],
                            op0=mybir.AluOpType.mult,
                            op1=mybir.AluOpType.add,
                        )
                        seng = nc.sync if (n * CT + ct) % 2 == 0 else nc.scalar
                        seng.dma_start(
                            out=y_d[b, ct * 128:(ct + 1) * 128, nsl], in_=st,
                        )

    nc.compile()
    return nc


def kernel(x: np.ndarray, gamma: np.ndarray) -> np.ndarray:
    x = np.ascontiguousarray(np.asarray(x, dtype=np.float32))
    gamma = np.ascontiguousarray(np.asarray(gamma, dtype=np.float32))
    B, Cc, H, W = x.shape
    xv = x.reshape(B, Cc, H * W)

    if "nc" not in _CACHE:
        _CACHE["nc"] = _build_nc()
    nc = _CACHE["nc"]

    in_maps = [
        {"x": xv[i * B_PER_CORE:(i + 1) * B_PER_CORE], "gamma": gamma}
        for i in range(N_CORES)
    ]
    res = run_bass_kernel_spmd(nc, in_maps, list(range(N_CORES)))
    y = np.concatenate([res.results[i]["y"] for i in range(N_CORES)], axis=0)
    return y.reshape(B, Cc, H, W).astype(np.float32)

